# revision 27
# baseline (speedup 1.0000x reference)
"""LoRA Linear kernel for Trainium2, 8-core hybrid-parallel (4 token groups
x 2 out-feature halves).

out = x @ W^T + b + 2.0 * ((x @ lora_B^T) @ lora_A^T)

Per-core strategy (core = token-group tg x out-half oh):
  - Host marshals x^T and W^T shards pre-tiled in bf16 so every DMA is
    128 partitions x 8KB-contiguous and the kernel needs ZERO on-chip
    transposes. All matmuls bf16 (fp32 PSUM accumulate).
  - LoRA: xr^T = lora_B @ x^T computed once per t-strip (32 K=128 MMs)
    during the first o-strip pass; each output psum group then gets one
    extra K=17 matmul [xr^T; ones] @ [2*A^T; b] that adds BOTH the
    rank-16 update and the bias. No DVE work on any matmul's critical
    path.
  - Sharding 4 token-groups x 2 out-halves minimizes host->device
    traffic (~270MB vs 1.2GB for pure tensor-parallel).

Main loop: 4 o-strips (512) x 4 t-strips (512) x 4 t-tiles (128) x 32 k.
Output is written bf16 and upcast to fp32 on the host.
"""

import numpy as np

N_CORES = 8
B_DIM, S_DIM, D_IN, D_OUT = 4, 2048, 4096, 4096
T = B_DIM * S_DIM            # 8192 tokens
TG = 4                       # token groups
OH = 2                       # out-feature halves
T_SH = T // TG               # 2048 tokens per core
O_SH = D_OUT // OH           # 2048 out features per core
R = 16
P = 128
KB = D_IN // P               # 32 k-blocks
NOS = O_SH // 512            # 4 o-strips
NTS = T_SH // 512            # 4 t-strips
NSUB = 4                     # sub-DMAs per strip (8 k-blocks each)
KSUB = KB // NSUB

_CACHE = {}


def _build_nc():
    import concourse.bacc as bacc
    import concourse.mybir as mybir
    import concourse.tile as tile

    F32 = mybir.dt.float32
    BF16 = mybir.dt.bfloat16

    nc = bacc.Bacc(target_bir_lowering=False)
    # host-tiled layouts (see _make_in_maps):
    #   xt[ts*128+p, kb*512+u] = x_sh[ts*512+u, kb*128+p]   (= x^T tiled)
    #   wt[os*128+p, kb*512+u] = W_sh[os*512+u, kb*128+p]   (= W^T tiled)
    #   bt[p, kb*16+r]         = lora_B[r, kb*128+p]        (= B^T tiled)
    #   laug = [2*A_sh^T ; b_sh]  [17, O_SH]
    xt_d = nc.dram_tensor("xt", [NTS * P, KB * 512], BF16, kind="ExternalInput")
    wt_d = nc.dram_tensor("wt", [NOS * P, KB * 512], BF16, kind="ExternalInput")
    # laug is zero-padded to a full 128-partition operand: rows 32-47 /
    # 64-79 / 96-111 hold copies of 2*A^T (one per xr partial group), row 0
    # holds b. The lora matmul contracts all 128 rows, summing the three
    # xr partials and the bias in one shot.
    bt_d = nc.dram_tensor("bt", [P, KB * R], BF16, kind="ExternalInput")
    laug_d = nc.dram_tensor("laug", [P, O_SH], BF16, kind="ExternalInput")
    out_d = nc.dram_tensor("out", [T_SH, O_SH], BF16, kind="ExternalOutput")

    out_t = out_d[:].rearrange("(tt p) o -> p tt o", p=P)  # [128, 16, 2048]

    with tile.TileContext(nc) as tc:
        with (
            tc.tile_pool(name="const", bufs=1) as const,
            tc.tile_pool(name="xin", bufs=3) as xin,
            tc.tile_pool(name="win", bufs=2) as win,
            tc.tile_pool(name="osb", bufs=3) as osb_pool,
            tc.tile_pool(name="ps_o", bufs=4, space="PSUM") as ps_o,
            tc.tile_pool(name="ps_r", bufs=2, space="PSUM") as ps_r,
        ):
            btT = const.tile([P, KB, R], BF16)   # B^T tiled [128, 32, 16]
            laug = const.tile([P, O_SH], BF16)   # 2*A^T at rows 32/64/96+, b at 0
            xrT = const.tile([P, T_SH], BF16)    # xr partials at 32/64/96+, ones at 0

            # rows 32-47 / 64-79 / 96-111 get the three packed-xr partial
            # evictions; row 0 is the bias-ones row; the rest stay 0.
            nc.any.memset(xrT, 0.0)
            nc.any.memset(xrT[0:1, :], 1.0)
            nc.sync.dma_start(btT, bt_d[:].rearrange("p (kb r) -> p kb r", kb=KB))

            def x_sub(xsb, ts, s):
                nc.sync.dma_start(
                    xsb[:, s * KSUB:(s + 1) * KSUB, :],
                    xt_d[ts * P:(ts + 1) * P,
                         s * KSUB * 512:(s + 1) * KSUB * 512].rearrange(
                        "p (kb u) -> p kb u", kb=KSUB
                    ),
                )

            def w_sub(wsb, osi, s):
                nc.sync.dma_start(
                    wsb[:, s * KSUB:(s + 1) * KSUB, :],
                    wt_d[osi * P:(osi + 1) * P,
                         s * KSUB * 512:(s + 1) * KSUB * 512].rearrange(
                        "p (kb u) -> p kb u", kb=KSUB
                    ),
                )

            # startup: interleave the first x strip and first W strip so
            # the xr prologue (needs x+btT) and the first main groups
            # (need x+W) both start as soon as their sub-strips land.
            xsb0 = xin.tile([P, KB, 512], BF16, tag="x")
            wsb0 = win.tile([P, KB, 512], BF16, tag="w")
            for s in range(NSUB):
                x_sub(xsb0, 0, s)
                w_sub(wsb0, 0, s)
            nc.sync.dma_start(laug, laug_d[:])

            for osi in range(NOS):
                if osi == 0:
                    wsb = wsb0
                else:
                    wsb = win.tile([P, KB, 512], BF16, tag="w")
                    for s in range(NSUB):
                        w_sub(wsb, osi, s)
                for ts in range(NTS):
                    if osi == 0 and ts == 0:
                        xsb = xsb0
                    else:
                        xsb = xin.tile([P, KB, 512], BF16, tag="x")
                        for s in range(NSUB):
                            x_sub(xsb, ts, s)
                    if osi == 0:
                        # xr^T = B @ x^T, col-tiled 3x concurrent: partial
                        # sums over kb-thirds land at psum partition groups
                        # 32/64/96; the lora matmul's replicated 2*A^T rows
                        # absorb the cross-group reduction for free.
                        # group j takes kb = j, j+3, j+6, ... so step q only
                        # needs kbs 3q..3q+2 (consecutive -> sub-DMA local)
                        psr = ps_r.tile([P, 512], F32, tag="psr")
                        splits = [(j, 32 + 32 * j, list(range(j, KB, 3)))
                                  for j in range(3)]
                        for q in range(11):
                            for j, base, kbs in splits:
                                if q >= len(kbs):
                                    continue
                                kb = kbs[q]
                                nc.tensor.matmul(
                                    psr[base:base + R, :],
                                    btT[:, kb, :],
                                    xsb[:, kb, :],
                                    start=(q == 0),
                                    stop=(q == len(kbs) - 1),
                                    tile_position=(0, base),
                                )
                        for _, base, _ in splits:
                            nc.vector.tensor_copy(
                                out=xrT[base:base + R, ts * 512:(ts + 1) * 512],
                                in_=psr[base:base + R, :],
                            )
                    for tt in range(4):
                        pso = ps_o.tile([P, 512], F32, tag="pso")
                        for kb in range(KB):
                            nc.tensor.matmul(
                                pso,
                                xsb[:, kb, tt * P:(tt + 1) * P],
                                wsb[:, kb, :],
                                start=(kb == 0),
                                stop=False,
                            )
                        # rank-16 lora + bias in one full-array matmul
                        # (zero-padded K: rows 0-15 xr, row 32 ones/bias)
                        nc.tensor.matmul(
                            pso,
                            xrT[:, ts * 512 + tt * P:ts * 512 + (tt + 1) * P],
                            laug[:, osi * 512:(osi + 1) * 512],
                            start=False,
                            stop=True,
                        )
                        osb = osb_pool.tile([P, 512], BF16, tag="osb")
                        nc.vector.tensor_copy(out=osb, in_=pso)
                        nc.scalar.dma_start(
                            out_t[:, ts * 4 + tt, osi * 512:(osi + 1) * 512], osb
                        )

    nc.compile()
    return nc


def _get_nc():
    if "nc" not in _CACHE:
        _CACHE["nc"] = _build_nc()
    return _CACHE["nc"]


def _make_in_maps(inputs):
    import ml_dtypes

    bf16 = ml_dtypes.bfloat16
    x, W, b, lora_A, lora_B = (
        inputs["x"], inputs["W"], inputs["b"], inputs["lora_A"], inputs["lora_B"]
    )
    x_flat = np.asarray(x, dtype=np.float32).reshape(T, D_IN)
    W = np.asarray(W, dtype=np.float32)
    b = np.asarray(b, dtype=np.float32)
    lora_A = np.asarray(lora_A, dtype=np.float32)
    lora_B = np.asarray(lora_B, dtype=np.float32)

    # B^T tiled: bt[p, kb*16+r] = B[r, kb*128+p]
    bt = np.ascontiguousarray(
        lora_B.T.reshape(KB, P, R).transpose(1, 0, 2).reshape(P, KB * R)
    ).astype(bf16)

    xts = []
    for tg in range(TG):
        xs = x_flat[tg * T_SH:(tg + 1) * T_SH]           # [2048, 4096]
        h = xs.reshape(NTS, 512, KB, P).transpose(0, 3, 2, 1).astype(bf16)
        xts.append(np.ascontiguousarray(h.reshape(NTS * P, KB * 512)))
    wts, laugs = [], []
    for oh in range(OH):
        ws = W[oh * O_SH:(oh + 1) * O_SH]                # [2048, 4096]
        h = ws.reshape(NOS, 512, KB, P).transpose(0, 3, 2, 1).astype(bf16)
        wts.append(np.ascontiguousarray(h.reshape(NOS * P, KB * 512)))
        laug = np.zeros((P, O_SH), dtype=np.float32)
        a2 = 2.0 * lora_A[oh * O_SH:(oh + 1) * O_SH].T
        for base in (32, 64, 96):
            laug[base:base + R] = a2
        laug[0] = b[oh * O_SH:(oh + 1) * O_SH]
        laugs.append(laug.astype(bf16))

    in_maps = []
    for c in range(N_CORES):
        tg, oh = divmod(c, OH)
        in_maps.append({
            "xt": xts[tg],
            "wt": wts[oh],
            "bt": bt,
            "laug": laugs[oh],
        })
    return in_maps


def kernel(x, W, b, lora_A, lora_B):
    from concourse.bass_utils import run_bass_kernel_spmd

    nc = _get_nc()
    in_maps = _make_in_maps(dict(x=x, W=W, b=b, lora_A=lora_A, lora_B=lora_B))
    res = run_bass_kernel_spmd(nc, in_maps, core_ids=list(range(N_CORES)))
    out = np.empty((T, D_OUT), dtype=np.float32)
    for c in range(N_CORES):
        tg, oh = divmod(c, OH)
        out[tg * T_SH:(tg + 1) * T_SH, oh * O_SH:(oh + 1) * O_SH] = (
            res.results[c]["out"].astype(np.float32)
        )
    return out.reshape(B_DIM, S_DIM, D_OUT)


# revision 32
# speedup vs baseline: 1.0046x; 1.0046x over previous
"""LoRA Linear kernel for Trainium2, 8-core hybrid-parallel (4 token groups
x 2 out-feature halves).

out = x @ W^T + b + 2.0 * ((x @ lora_B^T) @ lora_A^T)

Per-core strategy (core = token-group tg x out-half oh):
  - Host marshals x^T and W^T shards pre-tiled in bf16 so every DMA is
    128 partitions x 8KB-contiguous and the kernel needs ZERO on-chip
    transposes. All matmuls bf16 (fp32 PSUM accumulate).
  - LoRA: xr^T = lora_B @ x^T computed once per t-strip (32 K=128 MMs)
    during the first o-strip pass; each output psum group then gets one
    extra K=17 matmul [xr^T; ones] @ [2*A^T; b] that adds BOTH the
    rank-16 update and the bias. No DVE work on any matmul's critical
    path.
  - Sharding 4 token-groups x 2 out-halves minimizes host->device
    traffic (~270MB vs 1.2GB for pure tensor-parallel).

Main loop: 4 o-strips (512) x 4 t-strips (512) x 4 t-tiles (128) x 32 k.
Output is written bf16 and upcast to fp32 on the host.
"""

import numpy as np

N_CORES = 8
B_DIM, S_DIM, D_IN, D_OUT = 4, 2048, 4096, 4096
T = B_DIM * S_DIM            # 8192 tokens
TG = 4                       # token groups
OH = 2                       # out-feature halves
T_SH = T // TG               # 2048 tokens per core
O_SH = D_OUT // OH           # 2048 out features per core
R = 16
P = 128
KB = D_IN // P               # 32 k-blocks
NOS = O_SH // 512            # 4 o-strips
NTS = T_SH // 512            # 4 t-strips
NSUB = 4                     # sub-DMAs per strip (8 k-blocks each)
KSUB = KB // NSUB

_CACHE = {}


def _build_nc():
    import concourse.bacc as bacc
    import concourse.mybir as mybir
    import concourse.tile as tile

    F32 = mybir.dt.float32
    BF16 = mybir.dt.bfloat16

    nc = bacc.Bacc(target_bir_lowering=False)
    # host-tiled layouts (see _make_in_maps):
    #   xt[ts*128+p, kb*512+u] = x_sh[ts*512+u, kb*128+p]   (= x^T tiled)
    #   wt[os*128+p, kb*512+u] = W_sh[os*512+u, kb*128+p]   (= W^T tiled)
    #   bt[p, kb*16+r]         = lora_B[r, kb*128+p]        (= B^T tiled)
    #   laug = [2*A_sh^T ; b_sh]  [17, O_SH]
    xt_d = nc.dram_tensor("xt", [NTS * P, KB * 512], BF16, kind="ExternalInput")
    wt_d = nc.dram_tensor("wt", [NOS * P, KB * 512], BF16, kind="ExternalInput")
    # laug is zero-padded to a full 128-partition operand: rows 32-47 /
    # 64-79 / 96-111 hold copies of 2*A^T (one per xr partial group), row 0
    # holds b. The lora matmul contracts all 128 rows, summing the three
    # xr partials and the bias in one shot.
    bt_d = nc.dram_tensor("bt", [P, KB * R], BF16, kind="ExternalInput")
    laug_d = nc.dram_tensor("laug", [P, O_SH], BF16, kind="ExternalInput")
    ones_d = nc.dram_tensor("ones", [1, T_SH], BF16, kind="ExternalInput")
    out_d = nc.dram_tensor("out", [T_SH, O_SH], BF16, kind="ExternalOutput")

    out_t = out_d[:].rearrange("(tt p) o -> p tt o", p=P)  # [128, 16, 2048]

    with tile.TileContext(nc) as tc:
        with (
            tc.tile_pool(name="const", bufs=1) as const,
            tc.tile_pool(name="xin", bufs=3) as xin,
            tc.tile_pool(name="win", bufs=2) as win,
            tc.tile_pool(name="osb", bufs=3) as osb_pool,
            tc.tile_pool(name="ps_o", bufs=4, space="PSUM") as ps_o,
            tc.tile_pool(name="ps_r", bufs=2, space="PSUM") as ps_r,
        ):
            btT = const.tile([P, KB, R], BF16)   # B^T tiled [128, 32, 16]
            laug = const.tile([P, O_SH], BF16)   # 2*A^T at rows 32/64/96+, b at 0
            xrT = const.tile([P, T_SH], BF16)    # xr partials at 32/64/96+, ones at 0

            # rows 0-15 / 32-47 / 64-79 / 96-111 get the four packed-xr
            # partial evictions; row 16 is the bias-ones row (DMA-written —
            # compute-engine APs can't start at partition 16, DMA can);
            # the rest stay 0.
            nc.any.memset(xrT, 0.0)
            nc.sync.dma_start(xrT[R:R + 1, :], ones_d[:])
            nc.sync.dma_start(btT, bt_d[:].rearrange("p (kb r) -> p kb r", kb=KB))

            def x_sub(xsb, ts, s):
                nc.sync.dma_start(
                    xsb[:, s * KSUB:(s + 1) * KSUB, :],
                    xt_d[ts * P:(ts + 1) * P,
                         s * KSUB * 512:(s + 1) * KSUB * 512].rearrange(
                        "p (kb u) -> p kb u", kb=KSUB
                    ),
                )

            def w_sub(wsb, osi, s):
                nc.sync.dma_start(
                    wsb[:, s * KSUB:(s + 1) * KSUB, :],
                    wt_d[osi * P:(osi + 1) * P,
                         s * KSUB * 512:(s + 1) * KSUB * 512].rearrange(
                        "p (kb u) -> p kb u", kb=KSUB
                    ),
                )

            # startup: interleave the first x strip and first W strip so
            # the xr prologue (needs x+btT) and the first main groups
            # (need x+W) both start as soon as their sub-strips land.
            xsb0 = xin.tile([P, KB, 512], BF16, tag="x")
            wsb0 = win.tile([P, KB, 512], BF16, tag="w")
            for s in range(NSUB):
                x_sub(xsb0, 0, s)
                w_sub(wsb0, 0, s)
            nc.sync.dma_start(laug, laug_d[:])

            for osi in range(NOS):
                if osi == 0:
                    wsb = wsb0
                else:
                    wsb = win.tile([P, KB, 512], BF16, tag="w")
                    for s in range(NSUB):
                        w_sub(wsb, osi, s)
                for ts in range(NTS):
                    if osi == 0 and ts == 0:
                        xsb = xsb0
                    else:
                        xsb = xin.tile([P, KB, 512], BF16, tag="x")
                        for s in range(NSUB):
                            x_sub(xsb, ts, s)
                    if osi == 0:
                        # xr^T = B @ x^T, col-tiled 3x concurrent: partial
                        # sums over kb-thirds land at psum partition groups
                        # 32/64/96; the lora matmul's replicated 2*A^T rows
                        # absorb the cross-group reduction for free.
                        # group j takes kb = j, j+3, j+6, ... so step q only
                        # needs kbs 3q..3q+2 (consecutive -> sub-DMA local)
                        psr = ps_r.tile([P, 512], F32, tag="psr")
                        splits = [(j, 32 * j, list(range(j, KB, 4)))
                                  for j in range(4)]
                        for q in range(8):
                            for j, base, kbs in splits:
                                if q >= len(kbs):
                                    continue
                                kb = kbs[q]
                                nc.tensor.matmul(
                                    psr[base:base + R, :],
                                    btT[:, kb, :],
                                    xsb[:, kb, :],
                                    start=(q == 0),
                                    stop=(q == len(kbs) - 1),
                                    tile_position=(0, base),
                                )
                        for _, base, _ in splits:
                            nc.vector.tensor_copy(
                                out=xrT[base:base + R, ts * 512:(ts + 1) * 512],
                                in_=psr[base:base + R, :],
                            )
                    for tt in range(4):
                        pso = ps_o.tile([P, 512], F32, tag="pso")
                        for kb in range(KB):
                            nc.tensor.matmul(
                                pso,
                                xsb[:, kb, tt * P:(tt + 1) * P],
                                wsb[:, kb, :],
                                start=(kb == 0),
                                stop=False,
                            )
                        # rank-16 lora + bias in one full-array matmul
                        # (zero-padded K: rows 0-15 xr, row 32 ones/bias)
                        nc.tensor.matmul(
                            pso,
                            xrT[:, ts * 512 + tt * P:ts * 512 + (tt + 1) * P],
                            laug[:, osi * 512:(osi + 1) * 512],
                            start=False,
                            stop=True,
                        )
                        osb = osb_pool.tile([P, 512], BF16, tag="osb")
                        nc.vector.tensor_copy(out=osb, in_=pso)
                        nc.scalar.dma_start(
                            out_t[:, ts * 4 + tt, osi * 512:(osi + 1) * 512], osb
                        )

    nc.compile()
    return nc


def _get_nc():
    if "nc" not in _CACHE:
        _CACHE["nc"] = _build_nc()
    return _CACHE["nc"]


def _make_in_maps(inputs):
    import ml_dtypes

    bf16 = ml_dtypes.bfloat16
    x, W, b, lora_A, lora_B = (
        inputs["x"], inputs["W"], inputs["b"], inputs["lora_A"], inputs["lora_B"]
    )
    x_flat = np.asarray(x, dtype=np.float32).reshape(T, D_IN)
    W = np.asarray(W, dtype=np.float32)
    b = np.asarray(b, dtype=np.float32)
    lora_A = np.asarray(lora_A, dtype=np.float32)
    lora_B = np.asarray(lora_B, dtype=np.float32)

    # B^T tiled: bt[p, kb*16+r] = B[r, kb*128+p]
    bt = np.ascontiguousarray(
        lora_B.T.reshape(KB, P, R).transpose(1, 0, 2).reshape(P, KB * R)
    ).astype(bf16)

    xts = []
    for tg in range(TG):
        xs = x_flat[tg * T_SH:(tg + 1) * T_SH]           # [2048, 4096]
        h = xs.reshape(NTS, 512, KB, P).transpose(0, 3, 2, 1).astype(bf16)
        xts.append(np.ascontiguousarray(h.reshape(NTS * P, KB * 512)))
    wts, laugs = [], []
    for oh in range(OH):
        ws = W[oh * O_SH:(oh + 1) * O_SH]                # [2048, 4096]
        h = ws.reshape(NOS, 512, KB, P).transpose(0, 3, 2, 1).astype(bf16)
        wts.append(np.ascontiguousarray(h.reshape(NOS * P, KB * 512)))
        laug = np.zeros((P, O_SH), dtype=np.float32)
        a2 = 2.0 * lora_A[oh * O_SH:(oh + 1) * O_SH].T
        for base in (0, 32, 64, 96):
            laug[base:base + R] = a2
        laug[R] = b[oh * O_SH:(oh + 1) * O_SH]
        laugs.append(laug.astype(bf16))

    ones = np.ones((1, T_SH), dtype=np.float32).astype(bf16)
    in_maps = []
    for c in range(N_CORES):
        tg, oh = divmod(c, OH)
        in_maps.append({
            "xt": xts[tg],
            "wt": wts[oh],
            "bt": bt,
            "laug": laugs[oh],
            "ones": ones,
        })
    return in_maps


def kernel(x, W, b, lora_A, lora_B):
    from concourse.bass_utils import run_bass_kernel_spmd

    nc = _get_nc()
    in_maps = _make_in_maps(dict(x=x, W=W, b=b, lora_A=lora_A, lora_B=lora_B))
    res = run_bass_kernel_spmd(nc, in_maps, core_ids=list(range(N_CORES)))
    out = np.empty((T, D_OUT), dtype=np.float32)
    for c in range(N_CORES):
        tg, oh = divmod(c, OH)
        out[tg * T_SH:(tg + 1) * T_SH, oh * O_SH:(oh + 1) * O_SH] = (
            res.results[c]["out"].astype(np.float32)
        )
    return out.reshape(B_DIM, S_DIM, D_OUT)
